# revision 3
# baseline (speedup 1.0000x reference)
"""Trainium2 Bass kernel for nn_CausalFFNN (pairwise relu-MLP scores).

Computes: Hn = relu(relu(E@W1+b1)@W2+b2)
          logits[i,j] = relu(Hn[i]@Wa + Hn[j]@Wb + bp1) @ Wp2 + bp2
          out = softplus(logits), diag = 0
Sharding: i-rows split across 8 cores (128 rows each); weights + full E
replicated. Each core computes a (128, 1024) output slab.

Dataflow per core:
  - encoder (PE): H1T/HnT over all 1024 tokens (transposed layout), plus the
    128-row slab encoder producing A^T = (Hn_slab@Wa + bp1)^T.
  - C^T = (Hn@Wb)^T computed unreplicated -> SBUF f16 (CT), then replicated
    8x across partition groups by SBUF->SBUF DMAs into per-t CTS tiles
    (partition p = g*16+u holds C[j, t*16+u] for every group g).
  - main loop over t (16 h-chunks), r (4), q (4): R = relu(CTS_t + A-bias)
    on DVE/ActE, then matmul with a block-diagonal stationary whose 8 weight
    columns sit at col offset 8r inside col-group q, so the (t,r,q) strips
    accumulate into a single [128, 1024] PSUM region covering all 128 i's.
  - drain: one Softplus over the PSUM region; output DMA un-permutes rows
    (partition 32q+8r+g holds slab row i = g*16+r*4+q).
"""
import sys
import os
import tempfile
import numpy as np

os.environ["NEURON_COMPILE_CACHE_URL"] = tempfile.mkdtemp(prefix="neuron-cache-")

for _p in ("/opt/trn_rl_repo", "/root/.axon_site/_ro/trn_rl_repo"):
    if os.path.isdir(_p) and _p not in sys.path:
        sys.path.insert(0, _p)

N, D, HID = 1024, 512, 256
NCORE = 8
SLAB = N // NCORE          # 128 i-rows per core
P = 128
G = 8                      # i's per (r,q) strip
HSUB = HID // 16           # 16 h-components per chunk
NT = HID // HSUB           # 16 chunks
NR = 4                     # col sub-offset within a 32-col group
NQ = 4                     # col groups
NJT = 2                    # 512-wide j tiles per 1024
ACT_EVERY = 5              # 1 of every ACT_EVERY relu tiles goes to ScalarE

_CACHE = {}


def _build_program(repeat=1):
    import concourse.bacc as bacc
    import concourse.mybir as mybir
    from concourse.tile import TileContext

    F32 = mybir.dt.float32
    F32R = mybir.dt.float32r
    F16 = mybir.dt.float16
    AF = mybir.ActivationFunctionType
    ALU = mybir.AluOpType

    nc = bacc.Bacc("TRN2", target_bir_lowering=False, debug=False)

    # ---- DRAM parameters (per core; arrays supplied via in_maps) ----
    dET = nc.dram_tensor("ET", [D, N], F32R, kind="ExternalInput")
    dETs = nc.dram_tensor("ETs", [D, SLAB], F32R, kind="ExternalInput")
    dW1s = nc.dram_tensor("W1s", [P, 4 * HID], F32R, kind="ExternalInput")
    dW2s = nc.dram_tensor("W2s", [P, 2 * HID], F32R, kind="ExternalInput")
    dWas = nc.dram_tensor("Was", [P, 2 * HID], F32R, kind="ExternalInput")
    dWbs = nc.dram_tensor("Wbs", [P, 2 * HID], F32R, kind="ExternalInput")
    db1 = nc.dram_tensor("b1c", [P, 2], F32, kind="ExternalInput")
    db2 = nc.dram_tensor("b2c", [P, 2], F32, kind="ExternalInput")
    dbp1 = nc.dram_tensor("bp1c", [P, 2], F32, kind="ExternalInput")
    dbp2 = nc.dram_tensor("bp2c", [P, 1], F32, kind="ExternalInput")
    dWst = nc.dram_tensor("Wst", [P, NT * NR * 32], F16, kind="ExternalInput")
    dY = nc.dram_tensor("Y", [SLAB, N], F32, kind="ExternalOutput")

    with TileContext(nc) as tc:
        with tc.tile_pool(name="const", bufs=1) as cpool, \
             tc.tile_pool(name="work", bufs=1) as wpool, \
             tc.tile_pool(name="rpool", bufs=10) as rpool, \
             tc.tile_pool(name="ctspool", bufs=4) as ctspool, \
             tc.tile_pool(name="dpool", bufs=1, space="DRAM") as dpool:

            # ---------- load constants (small/urgent first) ----------
            W1s = cpool.tile([P, 4 * HID], F32R, tag="W1s")
            nc.sync.dma_start(W1s[:], dW1s.ap())
            W2s = cpool.tile([P, 2 * HID], F32R, tag="W2s")
            nc.sync.dma_start(W2s[:], dW2s.ap())
            Was = cpool.tile([P, 2 * HID], F32R, tag="Was")
            nc.sync.dma_start(Was[:], dWas.ap())
            Wbs = cpool.tile([P, 2 * HID], F32R, tag="Wbs")
            nc.sync.dma_start(Wbs[:], dWbs.ap())
            b1c = cpool.tile([P, 2], F32, tag="b1c")
            nc.sync.dma_start(b1c[:], db1.ap())
            b2c = cpool.tile([P, 2], F32, tag="b2c")
            nc.sync.dma_start(b2c[:], db2.ap())
            bp1c = cpool.tile([P, 2], F32, tag="bp1c")
            nc.sync.dma_start(bp1c[:], dbp1.ap())
            bp2c = cpool.tile([P, 1], F32, tag="bp2c")
            nc.sync.dma_start(bp2c[:], dbp2.ap())
            Wst = cpool.tile([P, NT * NR * 32], F16, tag="Wst")
            nc.sync.dma_start(Wst[:], dWst.ap())
            ETs = cpool.tile([P, 4 * SLAB], F32R, tag="ETs")
            for kd in range(4):
                nc.sync.dma_start(ETs[:, kd * SLAB:(kd + 1) * SLAB],
                                  dETs.ap()[kd * P:(kd + 1) * P, :])
            ET = cpool.tile([P, 4 * N], F32R, tag="ET")
            for kd in range(4):
                nc.sync.dma_start(ET[:, kd * N:(kd + 1) * N],
                                  dET.ap()[kd * P:(kd + 1) * P, :])

            ATd = dpool.tile([HID, SLAB], F32, tag="ATd")
            H1T = wpool.tile([P, 2 * N], F32R, tag="H1T")
            HnT = wpool.tile([P, 2 * N], F32R, tag="HnT")
            H1Ts = wpool.tile([P, 2 * SLAB], F32R, tag="H1Ts")
            HnTs = wpool.tile([P, 2 * SLAB], F32R, tag="HnTs")
            ATs = wpool.tile([P, 2 * SLAB], F32, tag="ATs")
            CT = wpool.tile([P, 2 * N], F16, tag="CT")
            BT = wpool.tile([P, NT * 16], F32, tag="BT")

            def compute_body():
                # ---------- encoder (transposed, f32r) ----------
                with tc.tile_pool(name="eps", bufs=4, space="PSUM") as pps:
                    # H1T = relu(W1^T @ E^T + b1)
                    for mh in range(2):
                        for jt in range(2):
                            ps = pps.tile([P, 512], F32, tag="eps")
                            for kd in range(4):
                                nc.tensor.matmul(
                                    ps[:],
                                    W1s[:, kd * HID + mh * P: kd * HID + (mh + 1) * P],
                                    ET[:, kd * N + jt * 512: kd * N + (jt + 1) * 512],
                                    start=(kd == 0), stop=(kd == 3))
                            dstv = H1T[:, mh * N + jt * 512: mh * N + (jt + 1) * 512]
                            if jt == 0:
                                nc.scalar.activation(dstv, ps[:], AF.Relu,
                                                     bias=b1c[:, mh:mh + 1])
                            else:
                                nc.vector.tensor_scalar(dstv, ps[:], b1c[:, mh:mh + 1],
                                                        0.0, ALU.add, ALU.max)
                    # HnT = relu(W2^T @ H1T + b2)
                    for mh in range(2):
                        for jt in range(2):
                            ps = pps.tile([P, 512], F32, tag="eps")
                            for kh in range(2):
                                nc.tensor.matmul(
                                    ps[:],
                                    W2s[:, kh * HID + mh * P: kh * HID + (mh + 1) * P],
                                    H1T[:, kh * N + jt * 512: kh * N + (jt + 1) * 512],
                                    start=(kh == 0), stop=(kh == 1))
                            dstv = HnT[:, mh * N + jt * 512: mh * N + (jt + 1) * 512]
                            if jt == 0:
                                nc.scalar.activation(dstv, ps[:], AF.Relu,
                                                     bias=b2c[:, mh:mh + 1])
                            else:
                                nc.vector.tensor_scalar(dstv, ps[:], b2c[:, mh:mh + 1],
                                                        0.0, ALU.add, ALU.max)
                    # CT = (Hn @ Wb)^T, unreplicated, f16
                    for mh in range(2):
                        for jt in range(2):
                            ps = pps.tile([P, 512], F32, tag="eps")
                            for kh in range(2):
                                nc.tensor.matmul(
                                    ps[:],
                                    Wbs[:, kh * HID + mh * P: kh * HID + (mh + 1) * P],
                                    HnT[:, kh * N + jt * 512: kh * N + (jt + 1) * 512],
                                    start=(kh == 0), stop=(kh == 1))
                            dst = CT[:, mh * N + jt * 512: mh * N + (jt + 1) * 512]
                            if jt == 0:
                                nc.scalar.copy(dst, ps[:])
                            else:
                                nc.vector.tensor_copy(dst, ps[:])
                    # slab encoder: H1Ts, HnTs, ATs (Nf = 128)
                    for mh in range(2):
                        ps = pps.tile([P, SLAB], F32, tag="sps")
                        for kd in range(4):
                            nc.tensor.matmul(
                                ps[:],
                                W1s[:, kd * HID + mh * P: kd * HID + (mh + 1) * P],
                                ETs[:, kd * SLAB:(kd + 1) * SLAB],
                                start=(kd == 0), stop=(kd == 3))
                        nc.scalar.activation(
                            H1Ts[:, mh * SLAB:(mh + 1) * SLAB],
                            ps[:], AF.Relu, bias=b1c[:, mh:mh + 1])
                    for mh in range(2):
                        ps = pps.tile([P, SLAB], F32, tag="sps")
                        for kh in range(2):
                            nc.tensor.matmul(
                                ps[:],
                                W2s[:, kh * HID + mh * P: kh * HID + (mh + 1) * P],
                                H1Ts[:, kh * SLAB:(kh + 1) * SLAB],
                                start=(kh == 0), stop=(kh == 1))
                        nc.scalar.activation(
                            HnTs[:, mh * SLAB:(mh + 1) * SLAB],
                            ps[:], AF.Relu, bias=b2c[:, mh:mh + 1])
                    for mh in range(2):
                        ps = pps.tile([P, SLAB], F32, tag="sps")
                        for kh in range(2):
                            nc.tensor.matmul(
                                ps[:],
                                Was[:, kh * HID + mh * P: kh * HID + (mh + 1) * P],
                                HnTs[:, kh * SLAB:(kh + 1) * SLAB],
                                start=(kh == 0), stop=(kh == 1))
                        nc.scalar.activation(
                            ATs[:, mh * SLAB:(mh + 1) * SLAB],
                            ps[:], AF.Identity, bias=bp1c[:, mh:mh + 1])

                # ---------- BT via DRAM bounce ----------
                for mh in range(2):
                    nc.sync.dma_start(ATd[mh * P:(mh + 1) * P, :],
                                      ATs[:, mh * SLAB:(mh + 1) * SLAB])
                # bias col c = r*4+q serves i's {g*16+c}
                atd_v = ATd[:].rearrange("(t u) (gg c) -> gg u t c",
                                           u=HSUB, gg=G)
                for g in range(G):
                    dst = BT[g * HSUB:(g + 1) * HSUB, :] \
                        .rearrange("u (t c) -> u t c", c=16)
                    nc.sync.dma_start(dst, atd_v[g])

                # ---------- pairwise main loop ----------
                with tc.tile_pool(name="mps", bufs=1, space="PSUM") as mpool:
                    PS = mpool.tile([P, N], F32, tag="PS")
                    for t in range(NT):
                        # replicate C rows (t*16..t*16+16) across the 8 groups
                        CTS = ctspool.tile([P, N], F16, tag="CTS")
                        sp = (t * HSUB) % P
                        mh = t // 8
                        for g in range(G):
                            nc.sync.dma_start(
                                CTS[g * HSUB:(g + 1) * HSUB, :],
                                CT[sp:sp + HSUB, mh * N:(mh + 1) * N])
                        for r in range(NR):
                            for q in range(NQ):
                                R = rpool.tile([P, N], F16, tag="R")
                                c = r * 4 + q
                                bias_col = BT[:, t * 16 + c: t * 16 + c + 1]
                                rel_idx = t * 16 + c
                                if rel_idx % ACT_EVERY == 0:
                                    nc.scalar.activation(R[:], CTS[:], AF.Relu,
                                                         bias=bias_col)
                                else:
                                    nc.vector.tensor_scalar(R[:], CTS[:], bias_col,
                                                            0.0, ALU.add, ALU.max)
                                for jt in range(NJT):
                                    nc.tensor.matmul(
                                        PS[32 * q:32 * q + 32,
                                           jt * 512:(jt + 1) * 512],
                                        Wst[:, (t * NR + r) * 32:(t * NR + r + 1) * 32],
                                        R[:, jt * 512:(jt + 1) * 512],
                                        start=(t == 0 and r == 0),
                                        stop=(t == NT - 1 and r == NR - 1),
                                        tile_position=(0, 32 * q))
                    # drain: softplus = ln(1 + exp(logits + bp2)); diag on host
                    ESB = wpool.tile([P, N], F32, tag="ESB")
                    nc.scalar.activation(ESB[:], PS[:, :], AF.Exp,
                                         bias=bp2c[:, 0:1])
                    OUT2 = wpool.tile([P, N], F32, tag="OUT2")
                    nc.scalar.activation(OUT2[:], ESB[:], AF.Ln, bias=1.0)
                    # partition 32q+8r+g holds slab row i = g*16+r*4+q
                    for r in range(NR):
                        for q in range(NQ):
                            c = r * 4 + q
                            dst = dY.ap().rearrange("(g c) j -> c g j", c=16)[c]
                            nc.sync.dma_start(
                                dst, OUT2[32 * q + 8 * r: 32 * q + 8 * r + G, :])

            if repeat == 1:
                compute_body()
            else:
                with tc.For_i(0, repeat, 1):
                    compute_body()

    nc.compile()
    return nc


def _prep_inputs(E, W1, b1, W2, b2, Wp1, bp1, Wp2, bp2):
    f32 = np.float32
    E = np.asarray(E, f32)
    W1 = np.asarray(W1, f32)
    b1 = np.asarray(b1, f32)
    W2 = np.asarray(W2, f32)
    b2 = np.asarray(b2, f32)
    Wp1 = np.asarray(Wp1, f32)
    bp1 = np.asarray(bp1, f32)
    Wp2 = np.asarray(Wp2, f32)
    bp2 = np.asarray(bp2, f32)

    ET = np.ascontiguousarray(E.T)                      # (512, 1024)
    W1s = np.ascontiguousarray(
        W1.reshape(4, P, HID).transpose(1, 0, 2).reshape(P, 4 * HID))
    W2s = np.ascontiguousarray(
        W2.reshape(2, P, HID).transpose(1, 0, 2).reshape(P, 2 * HID))
    Wa, Wb = Wp1[:HID], Wp1[HID:]
    Was = np.ascontiguousarray(
        Wa.reshape(2, P, HID).transpose(1, 0, 2).reshape(P, 2 * HID))
    Wbs = np.ascontiguousarray(
        Wb.reshape(2, P, HID).transpose(1, 0, 2).reshape(P, 2 * HID))
    b1c = np.ascontiguousarray(b1.reshape(2, P).T)
    b2c = np.ascontiguousarray(b2.reshape(2, P).T)
    bp1c = np.ascontiguousarray(bp1.reshape(2, P).T)

    # stationary: for (t, r), 8 weight cols at local offset 8r+g in a 32-col
    # block; row (g,u) -> w[t*16+u] on col 8r+g
    Wst = np.zeros((P, NT * NR * 32), np.float16)
    w = Wp2[:, 0]
    for t in range(NT):
        for r in range(NR):
            for g in range(G):
                for u in range(HSUB):
                    Wst[g * HSUB + u, (t * NR + r) * 32 + 8 * r + g] = \
                        w[t * HSUB + u]

    bp2c = np.full((P, 1), bp2[0], np.float32)
    common = {
        "ET": ET, "W1s": W1s, "W2s": W2s, "Was": Was, "Wbs": Wbs,
        "b1c": b1c, "b2c": b2c, "bp1c": bp1c, "bp2c": bp2c, "Wst": Wst,
    }
    in_maps = []
    for k in range(NCORE):
        m = dict(common)
        m["ETs"] = np.ascontiguousarray(E[k * SLAB:(k + 1) * SLAB, :].T)
        in_maps.append(m)
    return in_maps, float(bp2[0])


def kernel(E, W1, b1, W2, b2, Wp1, bp1, Wp2, bp2):
    from concourse.bass_utils import run_bass_kernel_spmd

    if "nc" not in _CACHE:
        _CACHE["nc"] = _build_program()
    nc = _CACHE["nc"]

    in_maps, _ = _prep_inputs(E, W1, b1, W2, b2, Wp1, bp1, Wp2, bp2)
    res = run_bass_kernel_spmd(nc, in_maps, list(range(NCORE)))
    slabs = [res.results[k]["Y"] for k in range(NCORE)]
    out = np.concatenate(slabs, axis=0)
    np.fill_diagonal(out, 0.0)
    return np.ascontiguousarray(out.astype(np.float32))


# revision 6
# speedup vs baseline: 1.5932x; 1.5932x over previous
"""Trainium2 Bass kernel for nn_CausalFFNN (pairwise relu-MLP scores).

Computes: Hn = relu(relu(E@W1+b1)@W2+b2)
          logits[i,j] = relu(Hn[i]@Wa + Hn[j]@Wb + bp1) @ Wp2 + bp2
          out = softplus(logits), diag = 0
Sharding: i-rows split across 8 cores (128 rows each); weights + full E
replicated. Each core computes a (128, 1024) output slab.

Dataflow per core:
  - encoder (PE): H1T/HnT over all 1024 tokens (transposed layout), plus the
    128-row slab encoder producing ATs = (Hn_slab@Wa + bp1)^T.
  - CT = (Hn@Wb)^T, computed in two 128-row h-chunks -> SBUF f16.
  - main loop over t (2 h-chunks of 128), i (128 slab rows, as r=0..31 x
    q=0..3 with i = 32q+r): R = relu(CT_chunk + ATs[:, t*128+i]) on DVE/ActE
    ([128, 1024] f16, partition = h-within-chunk), then two 512-wide matmuls
    with a single-column stationary w[t*128+u] at col offset r of col-group q
    accumulate logits row i into PSUM partition 32q+r = i.
  - drain: exp(logits + bp2) then ln(1+x) over the [128, 1024] PSUM region;
    one plain DMA writes the slab (natural row order). Diagonal zeroed on
    host.
"""
import sys
import os
import tempfile
import numpy as np

os.environ["NEURON_COMPILE_CACHE_URL"] = tempfile.mkdtemp(prefix="neuron-cache-")

for _p in ("/opt/trn_rl_repo", "/root/.axon_site/_ro/trn_rl_repo"):
    if os.path.isdir(_p) and _p not in sys.path:
        sys.path.insert(0, _p)

N, D, HID = 1024, 512, 256
NCORE = 8
SLAB = N // NCORE          # 128 i-rows per core
P = 128
NT = HID // P              # 2 h-chunks
NRR = 32                   # r: col sub-offset within a 32-col group
NQ = 4                     # q: col groups
NJT = 2                    # 512-wide j tiles per 1024
ACT_EVERY = 5              # 1 of every ACT_EVERY relu tiles goes to ScalarE

_CACHE = {}


def _build_program(repeat=1):
    import concourse.bacc as bacc
    import concourse.mybir as mybir
    from concourse.tile import TileContext

    F32 = mybir.dt.float32
    F32R = mybir.dt.float32r
    F16 = mybir.dt.float16
    AF = mybir.ActivationFunctionType
    ALU = mybir.AluOpType

    nc = bacc.Bacc("TRN2", target_bir_lowering=False, debug=False)

    # ---- DRAM parameters (per core; arrays supplied via in_maps) ----
    dET = nc.dram_tensor("ET", [D, N], F32R, kind="ExternalInput")
    dETs = nc.dram_tensor("ETs", [D, SLAB], F32R, kind="ExternalInput")
    dW1s = nc.dram_tensor("W1s", [P, 4 * HID], F32R, kind="ExternalInput")
    dW2s = nc.dram_tensor("W2s", [P, 2 * HID], F32R, kind="ExternalInput")
    dWas = nc.dram_tensor("Was", [P, 2 * HID], F32R, kind="ExternalInput")
    dWbs = nc.dram_tensor("Wbs", [P, 2 * HID], F32R, kind="ExternalInput")
    db1 = nc.dram_tensor("b1c", [P, 2], F32, kind="ExternalInput")
    db2 = nc.dram_tensor("b2c", [P, 2], F32, kind="ExternalInput")
    dbp1 = nc.dram_tensor("bp1c", [P, 2], F32, kind="ExternalInput")
    dbp2 = nc.dram_tensor("bp2c", [P, 1], F32, kind="ExternalInput")
    dWst = nc.dram_tensor("Wst", [P, NT * NRR * 32], F16, kind="ExternalInput")
    dY = nc.dram_tensor("Y", [SLAB, N], F32, kind="ExternalOutput")

    with TileContext(nc) as tc:
        with tc.tile_pool(name="const", bufs=1) as cpool, \
             tc.tile_pool(name="work", bufs=1) as wpool, \
             tc.tile_pool(name="rpool", bufs=10) as rpool:

            # ---------- load constants (small/urgent first) ----------
            W1s = cpool.tile([P, 4 * HID], F32R, tag="W1s")
            nc.sync.dma_start(W1s[:], dW1s.ap())
            W2s = cpool.tile([P, 2 * HID], F32R, tag="W2s")
            nc.sync.dma_start(W2s[:], dW2s.ap())
            Was = cpool.tile([P, 2 * HID], F32R, tag="Was")
            nc.sync.dma_start(Was[:], dWas.ap())
            Wbs = cpool.tile([P, 2 * HID], F32R, tag="Wbs")
            nc.sync.dma_start(Wbs[:], dWbs.ap())
            b1c = cpool.tile([P, 2], F32, tag="b1c")
            nc.sync.dma_start(b1c[:], db1.ap())
            b2c = cpool.tile([P, 2], F32, tag="b2c")
            nc.sync.dma_start(b2c[:], db2.ap())
            bp1c = cpool.tile([P, 2], F32, tag="bp1c")
            nc.sync.dma_start(bp1c[:], dbp1.ap())
            bp2c = cpool.tile([P, 1], F32, tag="bp2c")
            nc.sync.dma_start(bp2c[:], dbp2.ap())
            Wst = cpool.tile([P, NT * NRR * 32], F16, tag="Wst")
            nc.sync.dma_start(Wst[:], dWst.ap())
            ETs = cpool.tile([P, 4 * SLAB], F32R, tag="ETs")
            for kd in range(4):
                nc.sync.dma_start(ETs[:, kd * SLAB:(kd + 1) * SLAB],
                                  dETs.ap()[kd * P:(kd + 1) * P, :])
            ET = cpool.tile([P, 4 * N], F32R, tag="ET")
            for kd in range(4):
                nc.sync.dma_start(ET[:, kd * N:(kd + 1) * N],
                                  dET.ap()[kd * P:(kd + 1) * P, :])

            H1T = wpool.tile([P, 2 * N], F32R, tag="H1T")
            HnT = wpool.tile([P, 2 * N], F32R, tag="HnT")
            H1Ts = wpool.tile([P, 2 * SLAB], F32R, tag="H1Ts")
            HnTs = wpool.tile([P, 2 * SLAB], F32R, tag="HnTs")
            ATs = wpool.tile([P, 2 * SLAB], F32, tag="ATs")
            CT = wpool.tile([P, 2 * N], F16, tag="CT")

            def compute_body():
                with tc.tile_pool(name="eps", bufs=4, space="PSUM") as pps:
                    # slab encoder first (only needs ETs): H1Ts, HnTs, ATs
                    for mh in range(2):
                        ps = pps.tile([P, SLAB], F32, tag="sps")
                        for kd in range(4):
                            nc.tensor.matmul(
                                ps[:],
                                W1s[:, kd * HID + mh * P: kd * HID + (mh + 1) * P],
                                ETs[:, kd * SLAB:(kd + 1) * SLAB],
                                start=(kd == 0), stop=(kd == 3))
                        nc.scalar.activation(
                            H1Ts[:, mh * SLAB:(mh + 1) * SLAB],
                            ps[:], AF.Relu, bias=b1c[:, mh:mh + 1])
                    for mh in range(2):
                        ps = pps.tile([P, SLAB], F32, tag="sps")
                        for kh in range(2):
                            nc.tensor.matmul(
                                ps[:],
                                W2s[:, kh * HID + mh * P: kh * HID + (mh + 1) * P],
                                H1Ts[:, kh * SLAB:(kh + 1) * SLAB],
                                start=(kh == 0), stop=(kh == 1))
                        nc.scalar.activation(
                            HnTs[:, mh * SLAB:(mh + 1) * SLAB],
                            ps[:], AF.Relu, bias=b2c[:, mh:mh + 1])
                    for mh in range(2):
                        ps = pps.tile([P, SLAB], F32, tag="sps")
                        for kh in range(2):
                            nc.tensor.matmul(
                                ps[:],
                                Was[:, kh * HID + mh * P: kh * HID + (mh + 1) * P],
                                HnTs[:, kh * SLAB:(kh + 1) * SLAB],
                                start=(kh == 0), stop=(kh == 1))
                        nc.scalar.activation(
                            ATs[:, mh * SLAB:(mh + 1) * SLAB],
                            ps[:], AF.Identity, bias=bp1c[:, mh:mh + 1])
                    # H1T = relu(W1^T @ E^T + b1), kd-outer would need 4 live
                    # PSUM tiles; keep kd-inner (ET chunks arrive early anyway)
                    for mh in range(2):
                        for jt in range(2):
                            ps = pps.tile([P, 512], F32, tag="eps")
                            for kd in range(4):
                                nc.tensor.matmul(
                                    ps[:],
                                    W1s[:, kd * HID + mh * P: kd * HID + (mh + 1) * P],
                                    ET[:, kd * N + jt * 512: kd * N + (jt + 1) * 512],
                                    start=(kd == 0), stop=(kd == 3))
                            dstv = H1T[:, mh * N + jt * 512: mh * N + (jt + 1) * 512]
                            if jt == 0:
                                nc.scalar.activation(dstv, ps[:], AF.Relu,
                                                     bias=b1c[:, mh:mh + 1])
                            else:
                                nc.vector.tensor_scalar(dstv, ps[:], b1c[:, mh:mh + 1],
                                                        0.0, ALU.add, ALU.max)
                    # HnT = relu(W2^T @ H1T + b2)
                    for mh in range(2):
                        for jt in range(2):
                            ps = pps.tile([P, 512], F32, tag="eps")
                            for kh in range(2):
                                nc.tensor.matmul(
                                    ps[:],
                                    W2s[:, kh * HID + mh * P: kh * HID + (mh + 1) * P],
                                    H1T[:, kh * N + jt * 512: kh * N + (jt + 1) * 512],
                                    start=(kh == 0), stop=(kh == 1))
                            dstv = HnT[:, mh * N + jt * 512: mh * N + (jt + 1) * 512]
                            if jt == 0:
                                nc.scalar.activation(dstv, ps[:], AF.Relu,
                                                     bias=b2c[:, mh:mh + 1])
                            else:
                                nc.vector.tensor_scalar(dstv, ps[:], b2c[:, mh:mh + 1],
                                                        0.0, ALU.add, ALU.max)
                    # CT = (Hn @ Wb)^T, f16; chunk mh serves main-loop t = mh
                    for mh in range(2):
                        for jt in range(2):
                            ps = pps.tile([P, 512], F32, tag="eps")
                            for kh in range(2):
                                nc.tensor.matmul(
                                    ps[:],
                                    Wbs[:, kh * HID + mh * P: kh * HID + (mh + 1) * P],
                                    HnT[:, kh * N + jt * 512: kh * N + (jt + 1) * 512],
                                    start=(kh == 0), stop=(kh == 1))
                            dst = CT[:, mh * N + jt * 512: mh * N + (jt + 1) * 512]
                            if jt == 0:
                                nc.scalar.copy(dst, ps[:])
                            else:
                                nc.vector.tensor_copy(dst, ps[:])

                # ---------- pairwise main loop ----------
                with tc.tile_pool(name="mps", bufs=1, space="PSUM") as mpool:
                    PS = mpool.tile([P, N], F32, tag="PS")
                    for t in range(NT):
                        src = CT[:, t * N:(t + 1) * N]
                        for r in range(NRR):
                            for q in range(NQ):
                                i = 32 * q + r
                                R = rpool.tile([P, N], F16, tag="R")
                                bias_col = ATs[:, t * SLAB + i: t * SLAB + i + 1]
                                if (t * SLAB + i) % ACT_EVERY == 0:
                                    nc.scalar.activation(R[:], src, AF.Relu,
                                                         bias=bias_col)
                                else:
                                    nc.vector.tensor_scalar(R[:], src, bias_col,
                                                            0.0, ALU.add, ALU.max)
                                for jt in range(NJT):
                                    nc.tensor.matmul(
                                        PS[32 * q:32 * (q + 1),
                                           jt * 512:(jt + 1) * 512],
                                        Wst[:, (t * NRR + r) * 32:
                                               (t * NRR + r + 1) * 32],
                                        R[:, jt * 512:(jt + 1) * 512],
                                        start=(t == 0 and r == 0),
                                        stop=(t == NT - 1 and r == NRR - 1),
                                        tile_position=(0, 32 * q))
                    # drain: softplus = ln(1 + exp(logits + bp2)); diag on host
                    ESB = wpool.tile([P, N], F32, tag="ESB")
                    nc.scalar.activation(ESB[:], PS[:, :], AF.Exp,
                                         bias=bp2c[:, 0:1])
                    OUT2 = wpool.tile([P, N], F32, tag="OUT2")
                    nc.scalar.activation(OUT2[:], ESB[:], AF.Ln, bias=1.0)
                    nc.sync.dma_start(dY.ap(), OUT2[:])

            if repeat == 1:
                compute_body()
            else:
                with tc.For_i(0, repeat, 1):
                    compute_body()

    nc.compile()
    return nc


def _prep_inputs(E, W1, b1, W2, b2, Wp1, bp1, Wp2, bp2):
    f32 = np.float32
    E = np.asarray(E, f32)
    W1 = np.asarray(W1, f32)
    b1 = np.asarray(b1, f32)
    W2 = np.asarray(W2, f32)
    b2 = np.asarray(b2, f32)
    Wp1 = np.asarray(Wp1, f32)
    bp1 = np.asarray(bp1, f32)
    Wp2 = np.asarray(Wp2, f32)
    bp2 = np.asarray(bp2, f32)

    ET = np.ascontiguousarray(E.T)                      # (512, 1024)
    W1s = np.ascontiguousarray(
        W1.reshape(4, P, HID).transpose(1, 0, 2).reshape(P, 4 * HID))
    W2s = np.ascontiguousarray(
        W2.reshape(2, P, HID).transpose(1, 0, 2).reshape(P, 2 * HID))
    Wa, Wb = Wp1[:HID], Wp1[HID:]
    Was = np.ascontiguousarray(
        Wa.reshape(2, P, HID).transpose(1, 0, 2).reshape(P, 2 * HID))
    Wbs = np.ascontiguousarray(
        Wb.reshape(2, P, HID).transpose(1, 0, 2).reshape(P, 2 * HID))
    b1c = np.ascontiguousarray(b1.reshape(2, P).T)
    b2c = np.ascontiguousarray(b2.reshape(2, P).T)
    bp1c = np.ascontiguousarray(bp1.reshape(2, P).T)

    # stationary block per (t, r): [128, 32] with col r = w[t*128+u]
    Wst = np.zeros((P, NT * NRR * 32), np.float16)
    for t in range(NT):
        for r in range(NRR):
            Wst[:, (t * NRR + r) * 32 + r] = Wp2[t * P:(t + 1) * P, 0]

    bp2c = np.full((P, 1), bp2[0], np.float32)
    common = {
        "ET": ET, "W1s": W1s, "W2s": W2s, "Was": Was, "Wbs": Wbs,
        "b1c": b1c, "b2c": b2c, "bp1c": bp1c, "bp2c": bp2c, "Wst": Wst,
    }
    in_maps = []
    for k in range(NCORE):
        m = dict(common)
        m["ETs"] = np.ascontiguousarray(E[k * SLAB:(k + 1) * SLAB, :].T)
        in_maps.append(m)
    return in_maps, float(bp2[0])


def kernel(E, W1, b1, W2, b2, Wp1, bp1, Wp2, bp2):
    from concourse.bass_utils import run_bass_kernel_spmd

    if "nc" not in _CACHE:
        _CACHE["nc"] = _build_program()
    nc = _CACHE["nc"]

    in_maps, _ = _prep_inputs(E, W1, b1, W2, b2, Wp1, bp1, Wp2, bp2)
    res = run_bass_kernel_spmd(nc, in_maps, list(range(NCORE)))
    slabs = [res.results[k]["Y"] for k in range(NCORE)]
    out = np.concatenate(slabs, axis=0)
    np.fill_diagonal(out, 0.0)
    return np.ascontiguousarray(out.astype(np.float32))


# revision 7
# speedup vs baseline: 1.7289x; 1.0852x over previous
"""Trainium2 Bass kernel for nn_CausalFFNN (pairwise relu-MLP scores).

Computes: Hn = relu(relu(E@W1+b1)@W2+b2)
          logits[i,j] = relu(Hn[i]@Wa + Hn[j]@Wb + bp1) @ Wp2 + bp2
          out = softplus(logits), diag = 0
Sharding: i-rows split across 8 cores (128 rows each); weights + full E
replicated. Each core computes a (128, 1024) output slab.

Dataflow per core:
  - encoder (PE): H1T/HnT over all 1024 tokens (transposed layout), plus the
    128-row slab encoder producing ATs = (Hn_slab@Wa + bp1)^T.
  - CT = (Hn@Wb)^T, computed in two 128-row h-chunks -> SBUF f16.
  - main loop over t (2 h-chunks of 128), i (128 slab rows, as r=0..31 x
    q=0..3 with i = 32q+r): R = relu(CT_chunk + ATs[:, t*128+i]) on DVE/ActE
    ([128, 1024] f16, partition = h-within-chunk), then two 512-wide matmuls
    with a single-column stationary w[t*128+u] at col offset r of col-group q
    accumulate logits row i into PSUM partition 32q+r = i.
  - drain: exp(logits + bp2) then ln(1+x) over the [128, 1024] PSUM region;
    one plain DMA writes the slab (natural row order). Diagonal zeroed on
    host.
"""
import sys
import os
import tempfile
import numpy as np

os.environ["NEURON_COMPILE_CACHE_URL"] = tempfile.mkdtemp(prefix="neuron-cache-")

for _p in ("/opt/trn_rl_repo", "/root/.axon_site/_ro/trn_rl_repo"):
    if os.path.isdir(_p) and _p not in sys.path:
        sys.path.insert(0, _p)

N, D, HID = 1024, 512, 256
NCORE = 8
SLAB = N // NCORE          # 128 i-rows per core
P = 128
NT = HID // P              # 2 h-chunks
NRR = 32                   # r: col sub-offset within a 32-col group
NQ = 4                     # q: col groups
NJT = 2                    # 512-wide j tiles per 1024
ACT_EVERY = 5              # 1 of every ACT_EVERY relu tiles goes to ScalarE

_CACHE = {}


def _build_program(repeat=1):
    import concourse.bacc as bacc
    import concourse.mybir as mybir
    from concourse.tile import TileContext

    F32 = mybir.dt.float32
    F32R = mybir.dt.float32r
    F16 = mybir.dt.float16
    AF = mybir.ActivationFunctionType
    ALU = mybir.AluOpType

    nc = bacc.Bacc("TRN2", target_bir_lowering=False, debug=False)

    # ---- DRAM parameters (per core; arrays supplied via in_maps) ----
    dET = nc.dram_tensor("ET", [D, N], F32R, kind="ExternalInput")
    dETs = nc.dram_tensor("ETs", [D, SLAB], F32R, kind="ExternalInput")
    dW1s = nc.dram_tensor("W1s", [P, 4 * HID], F32R, kind="ExternalInput")
    dW2s = nc.dram_tensor("W2s", [P, 2 * HID], F32R, kind="ExternalInput")
    dWas = nc.dram_tensor("Was", [P, 2 * HID], F32R, kind="ExternalInput")
    dWbs = nc.dram_tensor("Wbs", [P, 2 * HID], F32R, kind="ExternalInput")
    db1 = nc.dram_tensor("b1c", [P, 2], F32, kind="ExternalInput")
    db2 = nc.dram_tensor("b2c", [P, 2], F32, kind="ExternalInput")
    dbp1 = nc.dram_tensor("bp1c", [P, 2], F32, kind="ExternalInput")
    dbp2 = nc.dram_tensor("bp2c", [P, 1], F32, kind="ExternalInput")
    dWst = nc.dram_tensor("Wst", [P, NT * NRR * 32], F16, kind="ExternalInput")
    dY = nc.dram_tensor("Y", [SLAB, N], F32, kind="ExternalOutput")

    with TileContext(nc) as tc:
        with tc.tile_pool(name="const", bufs=1) as cpool, \
             tc.tile_pool(name="work", bufs=1) as wpool, \
             tc.tile_pool(name="rpool", bufs=10) as rpool:

            # ---------- load constants (small/urgent first) ----------
            W1s = cpool.tile([P, 4 * HID], F32R, tag="W1s")
            nc.sync.dma_start(W1s[:], dW1s.ap())
            W2s = cpool.tile([P, 2 * HID], F32R, tag="W2s")
            nc.sync.dma_start(W2s[:], dW2s.ap())
            Was = cpool.tile([P, 2 * HID], F32R, tag="Was")
            nc.sync.dma_start(Was[:], dWas.ap())
            Wbs = cpool.tile([P, 2 * HID], F32R, tag="Wbs")
            nc.sync.dma_start(Wbs[:], dWbs.ap())
            b1c = cpool.tile([P, 2], F32, tag="b1c")
            nc.sync.dma_start(b1c[:], db1.ap())
            b2c = cpool.tile([P, 2], F32, tag="b2c")
            nc.sync.dma_start(b2c[:], db2.ap())
            bp1c = cpool.tile([P, 2], F32, tag="bp1c")
            nc.sync.dma_start(bp1c[:], dbp1.ap())
            bp2c = cpool.tile([P, 1], F32, tag="bp2c")
            nc.sync.dma_start(bp2c[:], dbp2.ap())
            Wst = cpool.tile([P, NT * NRR * 32], F16, tag="Wst")
            nc.sync.dma_start(Wst[:], dWst.ap())
            ETs = cpool.tile([P, 4 * SLAB], F32R, tag="ETs")
            for kd in range(4):
                nc.sync.dma_start(ETs[:, kd * SLAB:(kd + 1) * SLAB],
                                  dETs.ap()[kd * P:(kd + 1) * P, :])
            ET = cpool.tile([P, 4 * N], F32R, tag="ET")
            for kd in range(4):
                nc.sync.dma_start(ET[:, kd * N:(kd + 1) * N],
                                  dET.ap()[kd * P:(kd + 1) * P, :])

            H1T = wpool.tile([P, 2 * N], F32R, tag="H1T")
            HnT = wpool.tile([P, 2 * N], F32R, tag="HnT")
            H1Ts = wpool.tile([P, 2 * SLAB], F32R, tag="H1Ts")
            HnTs = wpool.tile([P, 2 * SLAB], F32R, tag="HnTs")
            ATs = wpool.tile([P, 2 * SLAB], F32, tag="ATs")
            CT = wpool.tile([P, 2 * N], F16, tag="CT")

            def compute_body():
                with tc.tile_pool(name="eps", bufs=4, space="PSUM") as pps:
                    # slab encoder first (only needs ETs): H1Ts, HnTs, ATs
                    for mh in range(2):
                        ps = pps.tile([P, SLAB], F32, tag="sps")
                        for kd in range(4):
                            nc.tensor.matmul(
                                ps[:],
                                W1s[:, kd * HID + mh * P: kd * HID + (mh + 1) * P],
                                ETs[:, kd * SLAB:(kd + 1) * SLAB],
                                start=(kd == 0), stop=(kd == 3))
                        nc.scalar.activation(
                            H1Ts[:, mh * SLAB:(mh + 1) * SLAB],
                            ps[:], AF.Relu, bias=b1c[:, mh:mh + 1])
                    for mh in range(2):
                        ps = pps.tile([P, SLAB], F32, tag="sps")
                        for kh in range(2):
                            nc.tensor.matmul(
                                ps[:],
                                W2s[:, kh * HID + mh * P: kh * HID + (mh + 1) * P],
                                H1Ts[:, kh * SLAB:(kh + 1) * SLAB],
                                start=(kh == 0), stop=(kh == 1))
                        nc.scalar.activation(
                            HnTs[:, mh * SLAB:(mh + 1) * SLAB],
                            ps[:], AF.Relu, bias=b2c[:, mh:mh + 1])
                    for mh in range(2):
                        ps = pps.tile([P, SLAB], F32, tag="sps")
                        for kh in range(2):
                            nc.tensor.matmul(
                                ps[:],
                                Was[:, kh * HID + mh * P: kh * HID + (mh + 1) * P],
                                HnTs[:, kh * SLAB:(kh + 1) * SLAB],
                                start=(kh == 0), stop=(kh == 1))
                        nc.scalar.activation(
                            ATs[:, mh * SLAB:(mh + 1) * SLAB],
                            ps[:], AF.Identity, bias=bp1c[:, mh:mh + 1])
                    # H1T = relu(W1^T @ E^T + b1), kd-outer would need 4 live
                    # PSUM tiles; keep kd-inner (ET chunks arrive early anyway)
                    for mh in range(2):
                        for jt in range(2):
                            ps = pps.tile([P, 512], F32, tag="eps")
                            for kd in range(4):
                                nc.tensor.matmul(
                                    ps[:],
                                    W1s[:, kd * HID + mh * P: kd * HID + (mh + 1) * P],
                                    ET[:, kd * N + jt * 512: kd * N + (jt + 1) * 512],
                                    start=(kd == 0), stop=(kd == 3))
                            dstv = H1T[:, mh * N + jt * 512: mh * N + (jt + 1) * 512]
                            if jt == 0:
                                nc.scalar.activation(dstv, ps[:], AF.Relu,
                                                     bias=b1c[:, mh:mh + 1])
                            else:
                                nc.vector.tensor_scalar(dstv, ps[:], b1c[:, mh:mh + 1],
                                                        0.0, ALU.add, ALU.max)
                    # HnT = relu(W2^T @ H1T + b2)
                    for mh in range(2):
                        for jt in range(2):
                            ps = pps.tile([P, 512], F32, tag="eps")
                            for kh in range(2):
                                nc.tensor.matmul(
                                    ps[:],
                                    W2s[:, kh * HID + mh * P: kh * HID + (mh + 1) * P],
                                    H1T[:, kh * N + jt * 512: kh * N + (jt + 1) * 512],
                                    start=(kh == 0), stop=(kh == 1))
                            dstv = HnT[:, mh * N + jt * 512: mh * N + (jt + 1) * 512]
                            if jt == 0:
                                nc.scalar.activation(dstv, ps[:], AF.Relu,
                                                     bias=b2c[:, mh:mh + 1])
                            else:
                                nc.vector.tensor_scalar(dstv, ps[:], b2c[:, mh:mh + 1],
                                                        0.0, ALU.add, ALU.max)
                    # CT = (Hn @ Wb)^T, f16; chunk mh serves main-loop t = mh
                    for mh in range(2):
                        for jt in range(2):
                            ps = pps.tile([P, 512], F32, tag="eps")
                            for kh in range(2):
                                nc.tensor.matmul(
                                    ps[:],
                                    Wbs[:, kh * HID + mh * P: kh * HID + (mh + 1) * P],
                                    HnT[:, kh * N + jt * 512: kh * N + (jt + 1) * 512],
                                    start=(kh == 0), stop=(kh == 1))
                            dst = CT[:, mh * N + jt * 512: mh * N + (jt + 1) * 512]
                            if jt == 0:
                                nc.scalar.copy(dst, ps[:])
                            else:
                                nc.vector.tensor_copy(dst, ps[:])

                # ---------- pairwise main loop ----------
                probe = os.environ.get("KERNEL_PROBE", "full")
                with tc.tile_pool(name="mps", bufs=1, space="PSUM") as mpool:
                    PS = mpool.tile([P, N], F32, tag="PS")
                    for t in range(NT):
                        src = CT[:, t * N:(t + 1) * N]
                        Rshared = None
                        for r in range(NRR):
                            for q in range(NQ):
                                i = 32 * q + r
                                do_relu = probe != "pe_only" or Rshared is None
                                do_mm = probe != "dve_only" or r == 0
                                if do_relu:
                                    R = rpool.tile([P, N], F16, tag="R")
                                    bias_col = ATs[:, t * SLAB + i: t * SLAB + i + 1]
                                    if (t * SLAB + i) % ACT_EVERY == 0:
                                        nc.scalar.activation(R[:], src, AF.Relu,
                                                             bias=bias_col)
                                    else:
                                        nc.vector.tensor_scalar(R[:], src, bias_col,
                                                                0.0, ALU.add, ALU.max)
                                    Rshared = R
                                else:
                                    R = Rshared
                                if not do_mm:
                                    continue
                                for jt in range(NJT):
                                    nc.tensor.matmul(
                                        PS[32 * q:32 * (q + 1),
                                           jt * 512:(jt + 1) * 512],
                                        Wst[:, (t * NRR + r) * 32:
                                               (t * NRR + r + 1) * 32],
                                        R[:, jt * 512:(jt + 1) * 512],
                                        start=(t == 0 and r == 0),
                                        stop=(t == NT - 1 and
                                              (r == NRR - 1 or probe == "dve_only")),
                                        tile_position=(0, 32 * q))
                    # drain: softplus = ln(1 + exp(logits + bp2)); diag on host
                    ESB = wpool.tile([P, N], F32, tag="ESB")
                    nc.scalar.activation(ESB[:], PS[:, :], AF.Exp,
                                         bias=bp2c[:, 0:1])
                    OUT2 = wpool.tile([P, N], F32, tag="OUT2")
                    nc.scalar.activation(OUT2[:], ESB[:], AF.Ln, bias=1.0)
                    nc.sync.dma_start(dY.ap(), OUT2[:])

            if repeat == 1:
                compute_body()
            else:
                with tc.For_i(0, repeat, 1):
                    compute_body()

    nc.compile()
    return nc


def _prep_inputs(E, W1, b1, W2, b2, Wp1, bp1, Wp2, bp2):
    f32 = np.float32
    E = np.asarray(E, f32)
    W1 = np.asarray(W1, f32)
    b1 = np.asarray(b1, f32)
    W2 = np.asarray(W2, f32)
    b2 = np.asarray(b2, f32)
    Wp1 = np.asarray(Wp1, f32)
    bp1 = np.asarray(bp1, f32)
    Wp2 = np.asarray(Wp2, f32)
    bp2 = np.asarray(bp2, f32)

    ET = np.ascontiguousarray(E.T)                      # (512, 1024)
    W1s = np.ascontiguousarray(
        W1.reshape(4, P, HID).transpose(1, 0, 2).reshape(P, 4 * HID))
    W2s = np.ascontiguousarray(
        W2.reshape(2, P, HID).transpose(1, 0, 2).reshape(P, 2 * HID))
    Wa, Wb = Wp1[:HID], Wp1[HID:]
    Was = np.ascontiguousarray(
        Wa.reshape(2, P, HID).transpose(1, 0, 2).reshape(P, 2 * HID))
    Wbs = np.ascontiguousarray(
        Wb.reshape(2, P, HID).transpose(1, 0, 2).reshape(P, 2 * HID))
    b1c = np.ascontiguousarray(b1.reshape(2, P).T)
    b2c = np.ascontiguousarray(b2.reshape(2, P).T)
    bp1c = np.ascontiguousarray(bp1.reshape(2, P).T)

    # stationary block per (t, r): [128, 32] with col r = w[t*128+u]
    Wst = np.zeros((P, NT * NRR * 32), np.float16)
    for t in range(NT):
        for r in range(NRR):
            Wst[:, (t * NRR + r) * 32 + r] = Wp2[t * P:(t + 1) * P, 0]

    bp2c = np.full((P, 1), bp2[0], np.float32)
    common = {
        "ET": ET, "W1s": W1s, "W2s": W2s, "Was": Was, "Wbs": Wbs,
        "b1c": b1c, "b2c": b2c, "bp1c": bp1c, "bp2c": bp2c, "Wst": Wst,
    }
    in_maps = []
    for k in range(NCORE):
        m = dict(common)
        m["ETs"] = np.ascontiguousarray(E[k * SLAB:(k + 1) * SLAB, :].T)
        in_maps.append(m)
    return in_maps, float(bp2[0])


def kernel(E, W1, b1, W2, b2, Wp1, bp1, Wp2, bp2):
    from concourse.bass_utils import run_bass_kernel_spmd

    if "nc" not in _CACHE:
        _CACHE["nc"] = _build_program()
    nc = _CACHE["nc"]

    in_maps, _ = _prep_inputs(E, W1, b1, W2, b2, Wp1, bp1, Wp2, bp2)
    res = run_bass_kernel_spmd(nc, in_maps, list(range(NCORE)))
    slabs = [res.results[k]["Y"] for k in range(NCORE)]
    out = np.concatenate(slabs, axis=0)
    np.fill_diagonal(out, 0.0)
    return np.ascontiguousarray(out.astype(np.float32))
